# revision 20
# baseline (speedup 1.0000x reference)
"""Trainium2 Bass kernel for nn_NucleusMoELayer (MoE routing layer).

Strategy (8 NeuronCores, SPMD via run_bass_kernel_spmd):
  - Expert-parallel: core c owns experts {2c, 2c+1}. Shared expert is
    token-parallel: core c processes tokens [c*1024, (c+1)*1024).
  - Host computes the router (fp32, exact top-k) and performs the
    dispatch gather / combine scatter-add; the device does every dense
    matmul (gate_up + down for 2 experts, shared SwiGLU MLP slice) in
    bf16 with fp32 PSUM accumulation, plus SwiGLU activation and
    gating application.
  - All DRAM operands are pre-packed on the host into SBUF-native
    [128, k-slabs, cols] tile layouts so every DMA is one large fully
    contiguous transfer (512KB..1MB).  Weight loads issue on the sync
    HWDGE ring, activation loads on the scalar HWDGE ring, so the
    initial loads stream in parallel and the PE starts within ~9us.
  - The first weight quarter of the shared expert runs k-outer across
    all 8 PSUM banks so each arriving 512-row chunk immediately unlocks
    32 matmuls (PE ramps at DMA pace instead of waiting for the full
    quarter).
  - Outputs are written back as bf16 per [128, 512] slice to keep the
    post-matmul tail short.
"""

import math
import os
import sys
from contextlib import ExitStack

import numpy as np

for _p in ("/opt/trn_rl_repo",):
    if _p not in sys.path and os.path.isdir(_p):
        sys.path.insert(0, _p)

# ---- problem dims (hardcoded per spec) ----
BS, SLEN, DIM = 2, 4096, 2048
INNER = 1024
E = 16
CAPACITY_FACTOR = 2.0
CAP = max(1, math.ceil(CAPACITY_FACTOR * SLEN / E))  # 512
ROUTE_SCALE = 1.0
NCORES = 8
EPC = E // NCORES            # experts per core = 2
NTOK = BS * SLEN             # 8192
TSH = NTOK // NCORES         # shared-expert tokens per core = 1024
TE = BS * CAP                # routed slots per expert = 1024
P = 128
T = TSH                      # tokens per unit (== TE)
TT = 512                     # token tile (PSUM free dim)
NT = T // TT                 # 2
FQ = 512                     # weight-slab width in f columns
KD = DIM // P                # 16 k-slabs over model dim
KI = INNER // P              # 8 k-slabs over inner dim
I2 = 2 * INNER
NQ1 = I2 // FQ               # 4 column-quarters, layer 1
NQ2 = DIM // FQ              # 4 column-quarters, layer 2
NXC = 4                      # x chunks (4 k-slabs each)

WARMUP = int(os.environ.get("KERNEL_WARMUP", "12"))
_BACKEND = os.environ.get("KERNEL_BACKEND", "bass")  # "bass" | "numpy"


# --------------------------------------------------------------------------
# Host-side routing (fp32, mirrors the reference semantics exactly)
# --------------------------------------------------------------------------
def _route(hidden_states_unmodulated, timestep, gate_w):
    """Returns (gti, gating) both shaped (E, BS, CAP), int64/fp32.

    gti holds flat token indices into (BS*SLEN); gating is normalized
    and scaled by ROUTE_SCALE. Uses jax on CPU with the exact reference
    op sequence so the selected indices bit-match the reference router.
    """
    try:
        return _route_jax(hidden_states_unmodulated, timestep, gate_w)
    except Exception:
        return _route_np(hidden_states_unmodulated, timestep, gate_w)


def _route_jax(hidden_states_unmodulated, timestep, gate_w):
    import jax
    import jax.numpy as jnp

    cpu = jax.devices("cpu")[0]
    with jax.default_device(cpu):
        hsu = jax.device_put(np.asarray(hidden_states_unmodulated), cpu)
        ts = jax.device_put(np.asarray(timestep), cpu)
        gw = jax.device_put(np.asarray(gate_w), cpu)
        t_exp = jnp.broadcast_to(ts[:, None, :], (BS, SLEN, DIM))
        router_input = jnp.concatenate([t_exp, hsu], axis=-1)
        logits = router_input @ gw
        scores = jax.nn.sigmoid(logits.astype(jnp.float32)).astype(logits.dtype)
        affinity = jnp.swapaxes(scores, 1, 2)  # (BS, E, SLEN)
        _, top_indices = jax.lax.top_k(affinity, CAP)
        gating = jnp.take_along_axis(affinity, top_indices, axis=-1)
        top_indices = np.asarray(top_indices).astype(np.int64)
        gating = np.asarray(gating).astype(np.float32)
    return _finish_route(top_indices, gating)


def _route_np(hidden_states_unmodulated, timestep, gate_w):
    hsu = hidden_states_unmodulated.reshape(BS, SLEN, DIM).astype(np.float32)
    t32 = timestep.astype(np.float32)
    gw = gate_w.astype(np.float32)
    logits = hsu.reshape(-1, DIM) @ gw[DIM:]
    logits = logits.reshape(BS, SLEN, E) + (t32 @ gw[:DIM])[:, None, :]
    scores = 1.0 / (1.0 + np.exp(-logits.astype(np.float32)))
    affinity = scores.transpose(0, 2, 1)  # (BS, E, SLEN)
    # exact top-k: descending by value, ties broken toward lower index
    top_indices = np.argsort(-affinity, axis=-1, kind="stable")[:, :, :CAP]
    top_indices = top_indices.astype(np.int64)
    gating = np.take_along_axis(affinity, top_indices, axis=-1)
    return _finish_route(top_indices, gating.astype(np.float32))


def _finish_route(top_indices, gating):
    batch_offsets = (np.arange(BS, dtype=np.int64) * SLEN)[:, None, None]
    gti = np.swapaxes(batch_offsets + top_indices, 0, 1)      # (E, BS, CAP)
    gating = np.swapaxes(gating, 0, 1)                        # (E, BS, CAP)
    sums = np.zeros((NTOK,), np.float32)
    np.add.at(sums, gti.reshape(-1), gating.reshape(-1))
    gating = gating / (sums[gti] + 1e-12)
    gating = gating * ROUTE_SCALE
    return gti, gating.astype(np.float32)


# --------------------------------------------------------------------------
# Host-side packing into SBUF-native tile layouts
# --------------------------------------------------------------------------
def _pack_w1(w):
    """[DIM, I2] -> [NQ1, 2, P, 8*FQ]; (q, s, p, j, c) <- w[s*1024+j*128+p,
    q*512+c]."""
    return np.ascontiguousarray(
        w.reshape(2, 8, P, NQ1, FQ).transpose(3, 0, 2, 1, 4)
        .reshape(NQ1, 2, P, 8 * FQ))


def _unpack_w1(w):
    return np.ascontiguousarray(
        w.reshape(NQ1, 2, P, 8, FQ).transpose(1, 3, 2, 0, 4)
        .reshape(DIM, I2))


def _pack_w2(w):
    """[INNER, DIM] -> [NQ2, P, KI*FQ]; (q, p, j, c) <- w[j*128+p,
    q*512+c]."""
    return np.ascontiguousarray(
        w.reshape(KI, P, NQ2, FQ).transpose(2, 1, 0, 3)
        .reshape(NQ2, P, KI * FQ))


def _unpack_w2(w):
    return np.ascontiguousarray(
        w.reshape(NQ2, P, KI, FQ).transpose(2, 1, 0, 3).reshape(INNER, DIM))


def _pack_x(xT):
    """[DIM, T] -> [NXC, P, 4*T]; (c, p, j, t) <- xT[c*512+j*128+p, t]."""
    return np.ascontiguousarray(
        xT.reshape(NXC, 4, P, T).transpose(0, 2, 1, 3).reshape(NXC, P, 4 * T))


def _unpack_x(xp):
    return np.ascontiguousarray(
        xp.reshape(NXC, P, 4, T).transpose(0, 2, 1, 3).reshape(DIM, T))


# --------------------------------------------------------------------------
# Device program (Bass/Tile)
# --------------------------------------------------------------------------
def _patch_tile_drain():
    """Split the Tile tail-drain's sem waits across standalone wait
    instructions: walrus CoreV3 codegen rejects instructions carrying
    more than 2 sync waits ("Too many sync wait commands")."""
    import concourse.tile as tile

    if getattr(tile.TileContext, "_drain_split_patched", False):
        return

    def _drain_and_barrier(self, tick_clock, wait_clock):
        nc = self.nc
        probe = nc.sync.nop()
        wait_clock.add_sem_waits(
            probe.ins, tile.ScopedClock({None: tick_clock.global_clock}))
        si = probe.ins.sync_info
        waits = list(si.on_wait or []) if si else []
        if len(waits) > 1:
            si.on_wait = waits[:1]
            byname = {h.name: h for h in self.sems.allocated().values()}
            for w in waits[1:]:
                assert w.wait_mode == "sem-ge-imm", w
                nc.sync.wait_ge(byname[w.ant_name], w.wait_value)
        nc.sync.drain()
        nc.all_engine_barrier()
        popped = nc._tile_sem_poison_stack.pop()
        assert popped is self._sem_poison
        nc.clear_and_free_semaphores(list(self.sems.allocated().values()))

    tile.TileContext._drain_and_barrier = _drain_and_barrier
    tile.TileContext._drain_split_patched = True


def _split_multi_waits(nc):
    """This walrus build caps embedded sync waits at 1 per instruction
    ("Too many sync wait commands"); move excess waits onto same-engine
    NoOp carriers placed immediately before the instruction."""
    from concourse import mybir

    n = 0
    for f in nc.m.functions:
        for bb in f.blocks:
            insts = bb.instructions
            i = 0
            while i < len(insts):
                inst = insts[i]
                si = inst.sync_info
                waits = list(si.on_wait or []) if si else []
                if len(waits) > 1:
                    for w in waits[:-1]:
                        nop = mybir.InstNoOp(name=f"I-wsplit{n}", ins=[], outs=[])
                        n += 1
                        nop.engine = inst.engine
                        nop.sync_info = mybir.SyncInfo(on_wait=[w], on_update=[])
                        insts.insert(i, nop)
                        i += 1
                    si.on_wait = waits[-1:]
                i += 1
    return nc


def _build_nc():
    import concourse.bass as bass
    import concourse.tile as tile
    from concourse import mybir

    _patch_tile_drain()

    BF = mybir.dt.bfloat16
    F32 = mybir.dt.float32
    Sigmoid = mybir.ActivationFunctionType.Sigmoid

    nc = bass.Bass()
    xs_p = nc.declare_dram_parameter("xs_p", [NXC, P, 4 * T], BF, isOutput=False)
    xr_p = nc.declare_dram_parameter("xr_p", [EPC, NXC, P, 4 * T], BF,
                                     isOutput=False)
    w1s_p = nc.declare_dram_parameter("w1s_p", [NQ1, 2, P, 8 * FQ], BF,
                                      isOutput=False)
    w2s_p = nc.declare_dram_parameter("w2s_p", [NQ2, P, KI * FQ], BF,
                                      isOutput=False)
    w1e_p = nc.declare_dram_parameter("w1e_p", [EPC, NQ1, 2, P, 8 * FQ], BF,
                                      isOutput=False)
    w2e_p = nc.declare_dram_parameter("w2e_p", [EPC, NQ2, P, KI * FQ], BF,
                                      isOutput=False)
    gat = nc.declare_dram_parameter("gat", [EPC, P, T], F32, isOutput=False)
    ys_p = nc.declare_dram_parameter("ys_p", [DIM, T], BF, isOutput=True)
    yr_p = nc.declare_dram_parameter("yr_p", [EPC, DIM, T], BF, isOutput=True)

    with tile.TileContext(nc) as tc, ExitStack() as ctx:
        w1pool = ctx.enter_context(tc.tile_pool(name="w1", bufs=5))
        w2pool = ctx.enter_context(tc.tile_pool(name="w2", bufs=3))
        xpool = ctx.enter_context(tc.tile_pool(name="x", bufs=2 * NXC))
        hidp = ctx.enter_context(tc.tile_pool(name="hid", bufs=2 + KI))
        hpool = ctx.enter_context(tc.tile_pool(name="h", bufs=2 + KI))
        outp = ctx.enter_context(tc.tile_pool(name="o", bufs=4))
        gpool = ctx.enter_context(tc.tile_pool(name="g", bufs=2))
        tmpp = ctx.enter_context(tc.tile_pool(name="tmp", bufs=4))
        psum = ctx.enter_context(tc.tile_pool(name="ps", bufs=8, space="PSUM"))

        def load_x(dram4, eng):
            """4 chunk tiles [P, 4, T] from [NXC, P, 4*T] dram.

            The shared-expert x goes on the scalar HWDGE ring (parallel
            with the weight preamble on sync); routed-expert x goes on
            the sync ring *behind* the earlier weights so its prefetch
            can't starve the urgent weight stream at packet round-robin.
            """
            xts = []
            for c in range(NXC):
                xt = xpool.tile([P, 4, T], BF, tag="x", name="xt")
                if c == 0 and eng is nc.scalar:
                    # split the very first chunk so the PE's first matmul
                    # only waits on 512KB instead of 1MB
                    eng.dma_start(out=xt[:, 0:2, :], in_=dram4[0][:, 0:2 * T])
                    eng.dma_start(out=xt[:, 2:4, :], in_=dram4[0][:, 2 * T:])
                else:
                    eng.dma_start(out=xt[:], in_=dram4[c])
                xts.append(xt)
            return xts

        def xslab(xts, kk, t):
            return xts[kk // 4][:, kk % 4, t * TT:(t + 1) * TT]

        def load_w1q(dram_q, split_first=False):
            """2 half tiles [P, 8, FQ] for one layer-1 quarter (sync)."""
            wts = []
            for s in range(2):
                wt = w1pool.tile([P, 8, FQ], BF, tag="w1", name="w1t")
                if s == 0 and split_first:
                    nc.sync.dma_start(out=wt[:, 0:4, :],
                                      in_=dram_q[0][:, 0:4 * FQ])
                    nc.sync.dma_start(out=wt[:, 4:8, :],
                                      in_=dram_q[0][:, 4 * FQ:])
                else:
                    nc.sync.dma_start(out=wt[:], in_=dram_q[s])
                wts.append(wt)
            return wts

        def w1slab(wts, kk, fi):
            return wts[kk // 8][:, kk % 8, fi * P:(fi + 1) * P]

        def layer1(w_dram, xts, evict, jouter_q0, q0_wts=None):
            for q in range(NQ1):
                if q == 0 and q0_wts is not None:
                    wts = q0_wts
                else:
                    wts = load_w1q(w_dram[q])
                if q == 0 and jouter_q0:
                    # k-outer over all 8 PSUM banks: each arriving
                    # (w subchunk, x chunk) pair unlocks 32 matmuls.
                    pss = [psum.tile([P, TT], F32, tag="ps", name="ps")
                           for _ in range(8)]
                    for kk in range(KD):
                        for fi in range(4):
                            for t in range(NT):
                                nc.tensor.matmul(
                                    pss[fi * NT + t][:],
                                    lhsT=w1slab(wts, kk, fi),
                                    rhs=xslab(xts, kk, t),
                                    start=(kk == 0), stop=(kk == KD - 1),
                                )
                    for fi in range(4):
                        for t in range(NT):
                            evict(fi, t, pss[fi * NT + t])
                else:
                    for fi in range(4):
                        pss = [psum.tile([P, TT], F32, tag="ps", name="ps")
                               for _ in range(NT)]
                        for kk in range(KD):
                            for t in range(NT):
                                nc.tensor.matmul(
                                    pss[t][:],
                                    lhsT=w1slab(wts, kk, fi),
                                    rhs=xslab(xts, kk, t),
                                    start=(kk == 0), stop=(kk == KD - 1),
                                )
                        for t in range(NT):
                            evict(q * 4 + fi, t, pss[t])

        def layer2(w_dram, h, evict):
            for q in range(NQ2):
                wt = w2pool.tile([P, KI, FQ], BF, tag="w2", name="w2t")
                nc.sync.dma_start(out=wt[:], in_=w_dram[q])
                for fi in range(4):
                    pss = [psum.tile([P, TT], F32, tag="ps", name="ps")
                           for _ in range(NT)]
                    # t-outer: the t=0 group finishes 8 matmuls early, so
                    # its evict + output DMA overlap the t=1 group (keeps
                    # the post-matmul tail short at the end of the kernel)
                    for t in range(NT):
                        for j in range(KI):
                            nc.tensor.matmul(
                                pss[t][:],
                                lhsT=wt[:, j, fi * P:(fi + 1) * P],
                                rhs=h[j][:, t * TT:(t + 1) * TT],
                                start=(j == 0), stop=(j == KI - 1),
                            )
                        evict(q * 4 + fi, t, pss[t])

        def unit(xts, w1_dram, w2_dram, out_dram, silu_first, grow=None,
                 jouter=False, q0_wts=None):
            """Full SwiGLU MLP in transposed space; out_dram [DIM, T] bf16."""
            hid = {}
            h = {}

            def evict_gu(f, t, ps):
                # silu(v) = v * sigmoid(v)
                if f < KI:  # first half of gate_up output
                    if f not in hid:
                        hid[f] = hidp.tile([P, T], BF, tag="hid", name="hid")
                    if silu_first:
                        tmp = tmpp.tile([P, TT], F32, tag="tmp", name="tmp")
                        nc.scalar.activation(tmp[:], ps[:], Sigmoid)
                        nc.vector.tensor_mul(
                            out=hid[f][:, t * TT:(t + 1) * TT],
                            in0=ps[:], in1=tmp[:])
                    else:
                        nc.scalar.copy(
                            out=hid[f][:, t * TT:(t + 1) * TT], in_=ps[:])
                else:       # second half
                    fg = f - KI
                    if fg not in h:
                        h[fg] = hpool.tile([P, T], BF, tag="h", name="h")
                    if silu_first:
                        nc.vector.tensor_mul(
                            out=h[fg][:, t * TT:(t + 1) * TT],
                            in0=hid[fg][:, t * TT:(t + 1) * TT],
                            in1=ps[:],
                        )
                    else:
                        tmp = tmpp.tile([P, TT], F32, tag="tmp", name="tmp")
                        nc.scalar.activation(tmp[:], ps[:], Sigmoid)
                        tmp2 = tmpp.tile([P, TT], F32, tag="tmp2", name="tmp2")
                        nc.vector.tensor_mul(
                            out=tmp2[:],
                            in0=hid[fg][:, t * TT:(t + 1) * TT],
                            in1=ps[:],
                        )
                        nc.vector.tensor_mul(
                            out=h[fg][:, t * TT:(t + 1) * TT],
                            in0=tmp2[:],
                            in1=tmp[:],
                        )

            layer1(w1_dram, xts, evict_gu, jouter, q0_wts=q0_wts)

            stage = {}

            def evict_dn(f, t, ps):
                if f not in stage:
                    stage[f] = outp.tile([P, T], BF, tag="o", name="stage")
                if grow is None:
                    nc.scalar.copy(
                        out=stage[f][:, t * TT:(t + 1) * TT], in_=ps[:])
                else:
                    nc.vector.tensor_mul(
                        out=stage[f][:, t * TT:(t + 1) * TT],
                        in0=ps[:],
                        in1=grow[:, t * TT:(t + 1) * TT],
                    )
                nc.sync.dma_start(
                    out=out_dram[f * P:(f + 1) * P, t * TT:(t + 1) * TT],
                    in_=stage[f][:, t * TT:(t + 1) * TT],
                )

            layer2(w2_dram, h, evict_dn)

        # ---- PE warm-up: throwaway matmuls keep the PE busy (and open the
        # HAM clock-gate, 1.2 -> 2.4 GHz) while the first chunks land.
        wu_a = tmpp.tile([P, P], BF, tag="wu_a", name="wu_a")
        wu_b = tmpp.tile([P, TT], BF, tag="wu_b", name="wu_b")
        nc.vector.memset(wu_a[:], 0.0)
        nc.vector.memset(wu_b[:], 0.0)
        wu_ps = psum.tile([P, TT], F32, tag="ps", name="wu_ps")
        for _ in range(WARMUP):
            nc.tensor.matmul(wu_ps[:], lhsT=wu_a[:], rhs=wu_b[:],
                             start=True, stop=True)

        # ---- shared expert on this core's token slice ----
        # Hand-ordered preamble: x chunk 0 streams on the scalar ring in
        # parallel with the sync ring, whose issue order matches the q0
        # k-outer consumption order exactly (w s0 halves, xs c1, w s1,
        # xs c2, xs c3), so the PE never waits once the first 1MB lands.
        xs = [xpool.tile([P, 4, T], BF, tag="x", name="xt")
              for _ in range(NXC)]
        wq0 = [w1pool.tile([P, 8, FQ], BF, tag="w1", name="w1t")
               for _ in range(2)]
        nc.scalar.dma_start(out=xs[0][:, 0:1, :], in_=xs_p[0][:, 0:T])
        nc.scalar.dma_start(out=xs[0][:, 1:2, :], in_=xs_p[0][:, T:2 * T])
        nc.scalar.dma_start(out=xs[0][:, 2:4, :], in_=xs_p[0][:, 2 * T:])
        nc.sync.dma_start(out=wq0[0][:, 0:2, :], in_=w1s_p[0][0][:, 0:2 * FQ])
        nc.sync.dma_start(out=wq0[0][:, 2:4, :], in_=w1s_p[0][0][:, 2 * FQ:4 * FQ])
        nc.sync.dma_start(out=wq0[0][:, 4:8, :], in_=w1s_p[0][0][:, 4 * FQ:])
        nc.sync.dma_start(out=xs[1][:], in_=xs_p[1])
        nc.sync.dma_start(out=wq0[1][:], in_=w1s_p[0][1])
        nc.sync.dma_start(out=xs[2][:], in_=xs_p[2])
        nc.sync.dma_start(out=xs[3][:], in_=xs_p[3])
        unit(xs, w1s_p, w2s_p, ys_p, silu_first=False, jouter=True,
             q0_wts=wq0)

        # ---- routed experts (2 per core) ----
        for e in range(EPC):
            xe = load_x(xr_p[e], nc.sync)
            grow = gpool.tile([P, T], F32, tag="g", name="grow")
            nc.scalar.dma_start(out=grow[:], in_=gat[e])
            unit(xe, w1e_p[e], w2e_p[e], yr_p[e], silu_first=True, grow=grow)

    return nc


# --------------------------------------------------------------------------
# Device execution wrappers
# --------------------------------------------------------------------------
def _make_in_maps(x_flat, gti, gating, gate_up_proj, down_proj,
                  shared_in_w, shared_out_w):
    import ml_dtypes

    bf16 = ml_dtypes.bfloat16

    w1s = _pack_w1(shared_in_w).astype(bf16)
    w2s = _pack_w2(shared_out_w).astype(bf16)
    w1e_all = [_pack_w1(gate_up_proj[e]).astype(bf16) for e in range(E)]
    w2e_all = [_pack_w2(down_proj[e]).astype(bf16) for e in range(E)]

    in_maps = []
    for c in range(NCORES):
        e0 = c * EPC
        xr = np.stack([
            _pack_x(np.ascontiguousarray(x_flat[gti[e].reshape(-1)].T))
            for e in range(e0, e0 + EPC)
        ])  # (EPC, NXC, P, 4*T)
        xs = _pack_x(np.ascontiguousarray(x_flat[c * TSH:(c + 1) * TSH].T))
        in_maps.append({
            "xs_p": xs.astype(bf16),
            "xr_p": xr.astype(bf16),
            "w1s_p": w1s,
            "w2s_p": w2s,
            "w1e_p": np.stack(w1e_all[e0:e0 + EPC]),
            "w2e_p": np.stack(w2e_all[e0:e0 + EPC]),
            "gat": np.ascontiguousarray(np.broadcast_to(
                gating[e0:e0 + EPC].reshape(EPC, 1, TE),
                (EPC, P, TE))).astype(np.float32),
        })
    return in_maps


def _run_numpy(in_maps):
    """Emulates the device math (bf16 inputs, fp32 accumulation)."""
    import ml_dtypes
    results = []
    for m in in_maps:
        def mlp(xp, w1p, w2p, silu_first):
            xT = _unpack_x(np.asarray(xp, np.float32))          # (DIM, T)
            wi = _unpack_w1(np.asarray(w1p, np.float32))        # (DIM, I2)
            wo = _unpack_w2(np.asarray(w2p, np.float32))        # (INNER, DIM)
            gu = wi.T @ xT                                       # (I2, T)
            a, b = gu[:INNER], gu[INNER:]
            silu = lambda v: v / (1.0 + np.exp(-v))
            h = (silu(a) * b) if silu_first else (a * silu(b))
            h = h.astype(ml_dtypes.bfloat16).astype(np.float32)
            return wo.T @ h                                      # (DIM, T)

        ys = mlp(m["xs_p"], m["w1s_p"], m["w2s_p"], False)
        yr = np.stack([
            mlp(m["xr_p"][e], m["w1e_p"][e], m["w2e_p"][e], True)
            * m["gat"][e][:1, :]
            for e in range(EPC)
        ])
        results.append({
            "ys_p": ys.astype(ml_dtypes.bfloat16),
            "yr_p": yr.astype(ml_dtypes.bfloat16),
        })
    return results, None


_NC_CACHE = {}


def _install_ntff_hook():
    """Provide antenv.axon_hooks (missing in this image) so
    run_bass_kernel_spmd(trace=True) can NTFF-profile via the axon .so."""
    import contextlib
    import ctypes
    import types

    name = "antenv.axon_hooks"
    if name in sys.modules:
        return
    try:
        import antenv.axon_hooks  # noqa: F401
        return
    except ImportError:
        pass
    so_path = "/opt/axon/libaxon_pjrt.so"
    if not os.path.exists(so_path):
        return
    lib = ctypes.CDLL(so_path)
    if not hasattr(lib, "axon_start_nrt_profile"):
        return
    lib.axon_start_nrt_profile.argtypes = [
        ctypes.POINTER(ctypes.c_int64), ctypes.c_size_t]
    lib.axon_start_nrt_profile.restype = ctypes.c_int64
    lib.axon_stop_nrt_profile.argtypes = [ctypes.c_char_p]
    lib.axon_stop_nrt_profile.restype = ctypes.c_int64

    @contextlib.contextmanager
    def _hook(output_dir, device_ids):
        import jax
        jax.devices()
        if device_ids:
            ids = (ctypes.c_int64 * len(device_ids))(*device_ids)
            rc = lib.axon_start_nrt_profile(ids, len(device_ids))
        else:
            rc = lib.axon_start_nrt_profile(None, 0)
        if rc != 0:
            raise RuntimeError(f"axon_start_nrt_profile rc={rc}")
        try:
            yield
        finally:
            n = lib.axon_stop_nrt_profile(str(output_dir).encode())
            print(f"profile: {n} file(s) written to {output_dir}",
                  file=sys.stderr)

    mod = types.ModuleType(name)
    mod._hook = _hook
    mod.set_axon_ntff_profile_hook = lambda h: setattr(mod, "_hook", h)
    mod.get_axon_ntff_profile_hook = lambda: mod._hook
    sys.modules[name] = mod


def _run_bass(in_maps):
    from concourse.bass_utils import run_bass_kernel_spmd

    if "nc" not in _NC_CACHE:
        _NC_CACHE["nc"] = _split_multi_waits(_build_nc())
    nc = _NC_CACHE["nc"]
    trace = os.environ.get("KERNEL_TRACE", "0") == "1"
    if trace:
        _install_ntff_hook()
    out = run_bass_kernel_spmd(nc, in_maps, list(range(NCORES)), trace=trace)
    if out.exec_time_ns is not None:
        print(f"HW exec time: {out.exec_time_ns} ns", flush=True)
        if out.mean_exec_time_ns is not None:
            print(f"HW mean exec time: {out.mean_exec_time_ns:.0f} ns", flush=True)
    return out.results, out.exec_time_ns


# --------------------------------------------------------------------------
# Public entry point
# --------------------------------------------------------------------------
def kernel(hidden_states, hidden_states_unmodulated, timestep, gate_w,
           gate_up_proj, down_proj, shared_in_w, shared_out_w):
    hidden_states = np.asarray(hidden_states, dtype=np.float32)
    x_flat = hidden_states.reshape(NTOK, DIM)

    gti, gating = _route(np.asarray(hidden_states_unmodulated),
                         np.asarray(timestep), np.asarray(gate_w))

    in_maps = _make_in_maps(
        x_flat, gti, gating,
        np.asarray(gate_up_proj, dtype=np.float32),
        np.asarray(down_proj, dtype=np.float32),
        np.asarray(shared_in_w, dtype=np.float32),
        np.asarray(shared_out_w, dtype=np.float32),
    )

    if _BACKEND == "numpy":
        results, _ = _run_numpy(in_maps)
    else:
        results, _ = _run_bass(in_maps)

    # ---- combine on host ----
    out_flat = np.empty((NTOK, DIM), np.float32)
    for c in range(NCORES):
        out_flat[c * TSH:(c + 1) * TSH] = np.asarray(
            results[c]["ys_p"], np.float32).T
    for c in range(NCORES):
        yr = np.asarray(results[c]["yr_p"], np.float32)  # (EPC, DIM, TE)
        for ei in range(EPC):
            e = c * EPC + ei
            rows = yr[ei].T  # (TE, DIM) in (b, slot) order
            for b in range(BS):
                idx = gti[e, b]
                out_flat[idx] += rows[b * CAP:(b + 1) * CAP]
    return out_flat.reshape(BS, SLEN, DIM)


# revision 22
# speedup vs baseline: 1.0025x; 1.0025x over previous
"""Trainium2 Bass kernel for nn_NucleusMoELayer (MoE routing layer).

Strategy (8 NeuronCores, SPMD via run_bass_kernel_spmd):
  - Expert-parallel: core c owns experts {2c, 2c+1}. Shared expert is
    token-parallel: core c processes tokens [c*1024, (c+1)*1024).
  - Host computes the router (fp32, exact top-k) and performs the
    dispatch gather / combine scatter-add; the device does every dense
    matmul (gate_up + down for 2 experts, shared SwiGLU MLP slice) in
    bf16 with fp32 PSUM accumulation, plus SwiGLU activation and
    gating application.
  - All DRAM operands are pre-packed on the host into SBUF-native
    [128, k-slabs, cols] tile layouts so every DMA is one large fully
    contiguous transfer (512KB..1MB).  Weight loads issue on the sync
    HWDGE ring, activation loads on the scalar HWDGE ring, so the
    initial loads stream in parallel and the PE starts within ~9us.
  - The first weight quarter of the shared expert runs k-outer across
    all 8 PSUM banks so each arriving 512-row chunk immediately unlocks
    32 matmuls (PE ramps at DMA pace instead of waiting for the full
    quarter).
  - Outputs are written back as bf16 per [128, 512] slice to keep the
    post-matmul tail short.
"""

import math
import os
import sys
from contextlib import ExitStack

import numpy as np

for _p in ("/opt/trn_rl_repo",):
    if _p not in sys.path and os.path.isdir(_p):
        sys.path.insert(0, _p)

# ---- problem dims (hardcoded per spec) ----
BS, SLEN, DIM = 2, 4096, 2048
INNER = 1024
E = 16
CAPACITY_FACTOR = 2.0
CAP = max(1, math.ceil(CAPACITY_FACTOR * SLEN / E))  # 512
ROUTE_SCALE = 1.0
NCORES = 8
EPC = E // NCORES            # experts per core = 2
NTOK = BS * SLEN             # 8192
TSH = NTOK // NCORES         # shared-expert tokens per core = 1024
TE = BS * CAP                # routed slots per expert = 1024
P = 128
T = TSH                      # tokens per unit (== TE)
TT = 512                     # token tile (PSUM free dim)
NT = T // TT                 # 2
FQ = 512                     # weight-slab width in f columns
KD = DIM // P                # 16 k-slabs over model dim
KI = INNER // P              # 8 k-slabs over inner dim
I2 = 2 * INNER
NQ1 = I2 // FQ               # 4 column-quarters, layer 1
NQ2 = DIM // FQ              # 4 column-quarters, layer 2
NXC = 4                      # x chunks (4 k-slabs each)

WARMUP = int(os.environ.get("KERNEL_WARMUP", "12"))
_BACKEND = os.environ.get("KERNEL_BACKEND", "bass")  # "bass" | "numpy"


# --------------------------------------------------------------------------
# Host-side routing (fp32, mirrors the reference semantics exactly)
# --------------------------------------------------------------------------
def _route(hidden_states_unmodulated, timestep, gate_w):
    """Returns (gti, gating) both shaped (E, BS, CAP), int64/fp32.

    gti holds flat token indices into (BS*SLEN); gating is normalized
    and scaled by ROUTE_SCALE. Uses jax on CPU with the exact reference
    op sequence so the selected indices bit-match the reference router.
    """
    try:
        return _route_jax(hidden_states_unmodulated, timestep, gate_w)
    except Exception:
        return _route_np(hidden_states_unmodulated, timestep, gate_w)


def _route_jax(hidden_states_unmodulated, timestep, gate_w):
    import jax
    import jax.numpy as jnp

    cpu = jax.devices("cpu")[0]
    with jax.default_device(cpu):
        hsu = jax.device_put(np.asarray(hidden_states_unmodulated), cpu)
        ts = jax.device_put(np.asarray(timestep), cpu)
        gw = jax.device_put(np.asarray(gate_w), cpu)
        t_exp = jnp.broadcast_to(ts[:, None, :], (BS, SLEN, DIM))
        router_input = jnp.concatenate([t_exp, hsu], axis=-1)
        logits = router_input @ gw
        scores = jax.nn.sigmoid(logits.astype(jnp.float32)).astype(logits.dtype)
        affinity = jnp.swapaxes(scores, 1, 2)  # (BS, E, SLEN)
        _, top_indices = jax.lax.top_k(affinity, CAP)
        gating = jnp.take_along_axis(affinity, top_indices, axis=-1)
        top_indices = np.asarray(top_indices).astype(np.int64)
        gating = np.asarray(gating).astype(np.float32)
    return _finish_route(top_indices, gating)


def _route_np(hidden_states_unmodulated, timestep, gate_w):
    hsu = hidden_states_unmodulated.reshape(BS, SLEN, DIM).astype(np.float32)
    t32 = timestep.astype(np.float32)
    gw = gate_w.astype(np.float32)
    logits = hsu.reshape(-1, DIM) @ gw[DIM:]
    logits = logits.reshape(BS, SLEN, E) + (t32 @ gw[:DIM])[:, None, :]
    scores = 1.0 / (1.0 + np.exp(-logits.astype(np.float32)))
    affinity = scores.transpose(0, 2, 1)  # (BS, E, SLEN)
    # exact top-k: descending by value, ties broken toward lower index
    top_indices = np.argsort(-affinity, axis=-1, kind="stable")[:, :, :CAP]
    top_indices = top_indices.astype(np.int64)
    gating = np.take_along_axis(affinity, top_indices, axis=-1)
    return _finish_route(top_indices, gating.astype(np.float32))


def _finish_route(top_indices, gating):
    batch_offsets = (np.arange(BS, dtype=np.int64) * SLEN)[:, None, None]
    gti = np.swapaxes(batch_offsets + top_indices, 0, 1)      # (E, BS, CAP)
    gating = np.swapaxes(gating, 0, 1)                        # (E, BS, CAP)
    sums = np.zeros((NTOK,), np.float32)
    np.add.at(sums, gti.reshape(-1), gating.reshape(-1))
    gating = gating / (sums[gti] + 1e-12)
    gating = gating * ROUTE_SCALE
    return gti, gating.astype(np.float32)


# --------------------------------------------------------------------------
# Host-side packing into SBUF-native tile layouts
# --------------------------------------------------------------------------
def _pack_w1(w):
    """[DIM, I2] -> [NQ1, 2, P, 8*FQ]; (q, s, p, j, c) <- w[s*1024+j*128+p,
    q*512+c]."""
    return np.ascontiguousarray(
        w.reshape(2, 8, P, NQ1, FQ).transpose(3, 0, 2, 1, 4)
        .reshape(NQ1, 2, P, 8 * FQ))


def _unpack_w1(w):
    return np.ascontiguousarray(
        w.reshape(NQ1, 2, P, 8, FQ).transpose(1, 3, 2, 0, 4)
        .reshape(DIM, I2))


def _pack_w2(w):
    """[INNER, DIM] -> [NQ2, P, KI*FQ]; (q, p, j, c) <- w[j*128+p,
    q*512+c]."""
    return np.ascontiguousarray(
        w.reshape(KI, P, NQ2, FQ).transpose(2, 1, 0, 3)
        .reshape(NQ2, P, KI * FQ))


def _unpack_w2(w):
    return np.ascontiguousarray(
        w.reshape(NQ2, P, KI, FQ).transpose(2, 1, 0, 3).reshape(INNER, DIM))


def _pack_x(xT):
    """[DIM, T] -> [NXC, P, 4*T]; (c, p, j, t) <- xT[c*512+j*128+p, t]."""
    return np.ascontiguousarray(
        xT.reshape(NXC, 4, P, T).transpose(0, 2, 1, 3).reshape(NXC, P, 4 * T))


def _unpack_x(xp):
    return np.ascontiguousarray(
        xp.reshape(NXC, P, 4, T).transpose(0, 2, 1, 3).reshape(DIM, T))


# --------------------------------------------------------------------------
# Device program (Bass/Tile)
# --------------------------------------------------------------------------
def _patch_tile_drain():
    """Split the Tile tail-drain's sem waits across standalone wait
    instructions: walrus CoreV3 codegen rejects instructions carrying
    more than 2 sync waits ("Too many sync wait commands")."""
    import concourse.tile as tile

    if getattr(tile.TileContext, "_drain_split_patched", False):
        return

    def _drain_and_barrier(self, tick_clock, wait_clock):
        nc = self.nc
        probe = nc.sync.nop()
        wait_clock.add_sem_waits(
            probe.ins, tile.ScopedClock({None: tick_clock.global_clock}))
        si = probe.ins.sync_info
        waits = list(si.on_wait or []) if si else []
        if len(waits) > 1:
            si.on_wait = waits[:1]
            byname = {h.name: h for h in self.sems.allocated().values()}
            for w in waits[1:]:
                assert w.wait_mode == "sem-ge-imm", w
                nc.sync.wait_ge(byname[w.ant_name], w.wait_value)
        nc.sync.drain()
        nc.all_engine_barrier()
        popped = nc._tile_sem_poison_stack.pop()
        assert popped is self._sem_poison
        nc.clear_and_free_semaphores(list(self.sems.allocated().values()))

    tile.TileContext._drain_and_barrier = _drain_and_barrier
    tile.TileContext._drain_split_patched = True


def _strip_entry_barrier(nc):
    """Drop the prologue (block-0) all-engine barrier.  It orders the
    SWDGE descriptor-scratch memsets (gpsimd) against every other
    engine, but this kernel issues all DMAs via the HWDGE rings, which
    don't touch that scratch; the only SWDGE users are the gpsimd
    dma_reset/sem_clear at exit, which are program-ordered after the
    memsets on the same engine.  The barrier is self-resetting over
    dedicated barrier_* semaphores, so removing the complete set leaves
    every later barrier's semaphore accounting intact.  Net effect: the
    first weight/activation DMAs issue ~1.3us earlier."""
    from concourse import mybir

    bb = nc.m.functions[0].blocks[0]
    keep = [
        i for i in bb.instructions
        if not isinstance(i, (mybir.InstDrain, mybir.InstEventSemaphore))
    ]
    del bb.instructions[:]
    bb.instructions.extend(keep)
    return nc


def _split_multi_waits(nc):
    """This walrus build caps embedded sync waits at 1 per instruction
    ("Too many sync wait commands"); move excess waits onto same-engine
    NoOp carriers placed immediately before the instruction."""
    from concourse import mybir

    n = 0
    for f in nc.m.functions:
        for bb in f.blocks:
            insts = bb.instructions
            i = 0
            while i < len(insts):
                inst = insts[i]
                si = inst.sync_info
                waits = list(si.on_wait or []) if si else []
                if len(waits) > 1:
                    for w in waits[:-1]:
                        nop = mybir.InstNoOp(name=f"I-wsplit{n}", ins=[], outs=[])
                        n += 1
                        nop.engine = inst.engine
                        nop.sync_info = mybir.SyncInfo(on_wait=[w], on_update=[])
                        insts.insert(i, nop)
                        i += 1
                    si.on_wait = waits[-1:]
                i += 1
    return nc


def _build_nc():
    import concourse.bass as bass
    import concourse.tile as tile
    from concourse import mybir

    _patch_tile_drain()

    BF = mybir.dt.bfloat16
    F32 = mybir.dt.float32
    Sigmoid = mybir.ActivationFunctionType.Sigmoid

    nc = bass.Bass()
    xs_p = nc.declare_dram_parameter("xs_p", [NXC, P, 4 * T], BF, isOutput=False)
    xr_p = nc.declare_dram_parameter("xr_p", [EPC, NXC, P, 4 * T], BF,
                                     isOutput=False)
    w1s_p = nc.declare_dram_parameter("w1s_p", [NQ1, 2, P, 8 * FQ], BF,
                                      isOutput=False)
    w2s_p = nc.declare_dram_parameter("w2s_p", [NQ2, P, KI * FQ], BF,
                                      isOutput=False)
    w1e_p = nc.declare_dram_parameter("w1e_p", [EPC, NQ1, 2, P, 8 * FQ], BF,
                                      isOutput=False)
    w2e_p = nc.declare_dram_parameter("w2e_p", [EPC, NQ2, P, KI * FQ], BF,
                                      isOutput=False)
    gat = nc.declare_dram_parameter("gat", [EPC, P, T], F32, isOutput=False)
    ys_p = nc.declare_dram_parameter("ys_p", [DIM, T], BF, isOutput=True)
    yr_p = nc.declare_dram_parameter("yr_p", [EPC, DIM, T], BF, isOutput=True)

    with tile.TileContext(nc) as tc, ExitStack() as ctx:
        w1pool = ctx.enter_context(tc.tile_pool(name="w1", bufs=5))
        w2pool = ctx.enter_context(tc.tile_pool(name="w2", bufs=3))
        xpool = ctx.enter_context(tc.tile_pool(name="x", bufs=2 * NXC))
        hidp = ctx.enter_context(tc.tile_pool(name="hid", bufs=2 + KI))
        hpool = ctx.enter_context(tc.tile_pool(name="h", bufs=2 + KI))
        outp = ctx.enter_context(tc.tile_pool(name="o", bufs=4))
        gpool = ctx.enter_context(tc.tile_pool(name="g", bufs=2))
        tmpp = ctx.enter_context(tc.tile_pool(name="tmp", bufs=4))
        psum = ctx.enter_context(tc.tile_pool(name="ps", bufs=8, space="PSUM"))

        def load_x(dram4, eng):
            """4 chunk tiles [P, 4, T] from [NXC, P, 4*T] dram.

            The shared-expert x goes on the scalar HWDGE ring (parallel
            with the weight preamble on sync); routed-expert x goes on
            the sync ring *behind* the earlier weights so its prefetch
            can't starve the urgent weight stream at packet round-robin.
            """
            xts = []
            for c in range(NXC):
                xt = xpool.tile([P, 4, T], BF, tag="x", name="xt")
                if c == 0 and eng is nc.scalar:
                    # split the very first chunk so the PE's first matmul
                    # only waits on 512KB instead of 1MB
                    eng.dma_start(out=xt[:, 0:2, :], in_=dram4[0][:, 0:2 * T])
                    eng.dma_start(out=xt[:, 2:4, :], in_=dram4[0][:, 2 * T:])
                else:
                    eng.dma_start(out=xt[:], in_=dram4[c])
                xts.append(xt)
            return xts

        def xslab(xts, kk, t):
            return xts[kk // 4][:, kk % 4, t * TT:(t + 1) * TT]

        def load_w1q(dram_q, split_first=False):
            """2 half tiles [P, 8, FQ] for one layer-1 quarter (sync)."""
            wts = []
            for s in range(2):
                wt = w1pool.tile([P, 8, FQ], BF, tag="w1", name="w1t")
                if s == 0 and split_first:
                    nc.sync.dma_start(out=wt[:, 0:4, :],
                                      in_=dram_q[0][:, 0:4 * FQ])
                    nc.sync.dma_start(out=wt[:, 4:8, :],
                                      in_=dram_q[0][:, 4 * FQ:])
                else:
                    nc.sync.dma_start(out=wt[:], in_=dram_q[s])
                wts.append(wt)
            return wts

        def w1slab(wts, kk, fi):
            return wts[kk // 8][:, kk % 8, fi * P:(fi + 1) * P]

        def layer1(w_dram, xts, evict, jouter_q0, q0_wts=None):
            for q in range(NQ1):
                if q == 0 and q0_wts is not None:
                    wts = q0_wts
                else:
                    wts = load_w1q(w_dram[q])
                if q == 0 and jouter_q0:
                    # k-outer over all 8 PSUM banks: each arriving
                    # (w subchunk, x chunk) pair unlocks 32 matmuls.
                    pss = [psum.tile([P, TT], F32, tag="ps", name="ps")
                           for _ in range(8)]
                    for kk in range(KD):
                        for fi in range(4):
                            for t in range(NT):
                                nc.tensor.matmul(
                                    pss[fi * NT + t][:],
                                    lhsT=w1slab(wts, kk, fi),
                                    rhs=xslab(xts, kk, t),
                                    start=(kk == 0), stop=(kk == KD - 1),
                                )
                    for fi in range(4):
                        for t in range(NT):
                            evict(fi, t, pss[fi * NT + t])
                else:
                    for fi in range(4):
                        pss = [psum.tile([P, TT], F32, tag="ps", name="ps")
                               for _ in range(NT)]
                        for kk in range(KD):
                            for t in range(NT):
                                nc.tensor.matmul(
                                    pss[t][:],
                                    lhsT=w1slab(wts, kk, fi),
                                    rhs=xslab(xts, kk, t),
                                    start=(kk == 0), stop=(kk == KD - 1),
                                )
                        for t in range(NT):
                            evict(q * 4 + fi, t, pss[t])

        def layer2(w_dram, h, evict):
            for q in range(NQ2):
                wt = w2pool.tile([P, KI, FQ], BF, tag="w2", name="w2t")
                nc.sync.dma_start(out=wt[:], in_=w_dram[q])
                for fi in range(4):
                    pss = [psum.tile([P, TT], F32, tag="ps", name="ps")
                           for _ in range(NT)]
                    # t-outer: the t=0 group finishes 8 matmuls early, so
                    # its evict + output DMA overlap the t=1 group (keeps
                    # the post-matmul tail short at the end of the kernel)
                    for t in range(NT):
                        for j in range(KI):
                            nc.tensor.matmul(
                                pss[t][:],
                                lhsT=wt[:, j, fi * P:(fi + 1) * P],
                                rhs=h[j][:, t * TT:(t + 1) * TT],
                                start=(j == 0), stop=(j == KI - 1),
                            )
                        evict(q * 4 + fi, t, pss[t])

        def unit(xts, w1_dram, w2_dram, out_dram, silu_first, grow=None,
                 jouter=False, q0_wts=None):
            """Full SwiGLU MLP in transposed space; out_dram [DIM, T] bf16."""
            hid = {}
            h = {}

            def evict_gu(f, t, ps):
                # silu(v) = v * sigmoid(v)
                if f < KI:  # first half of gate_up output
                    if f not in hid:
                        hid[f] = hidp.tile([P, T], BF, tag="hid", name="hid")
                    if silu_first:
                        tmp = tmpp.tile([P, TT], F32, tag="tmp", name="tmp")
                        nc.scalar.activation(tmp[:], ps[:], Sigmoid)
                        nc.vector.tensor_mul(
                            out=hid[f][:, t * TT:(t + 1) * TT],
                            in0=ps[:], in1=tmp[:])
                    else:
                        nc.scalar.copy(
                            out=hid[f][:, t * TT:(t + 1) * TT], in_=ps[:])
                else:       # second half
                    fg = f - KI
                    if fg not in h:
                        h[fg] = hpool.tile([P, T], BF, tag="h", name="h")
                    if silu_first:
                        nc.vector.tensor_mul(
                            out=h[fg][:, t * TT:(t + 1) * TT],
                            in0=hid[fg][:, t * TT:(t + 1) * TT],
                            in1=ps[:],
                        )
                    else:
                        tmp = tmpp.tile([P, TT], F32, tag="tmp", name="tmp")
                        nc.scalar.activation(tmp[:], ps[:], Sigmoid)
                        tmp2 = tmpp.tile([P, TT], F32, tag="tmp2", name="tmp2")
                        nc.vector.tensor_mul(
                            out=tmp2[:],
                            in0=hid[fg][:, t * TT:(t + 1) * TT],
                            in1=ps[:],
                        )
                        nc.vector.tensor_mul(
                            out=h[fg][:, t * TT:(t + 1) * TT],
                            in0=tmp2[:],
                            in1=tmp[:],
                        )

            layer1(w1_dram, xts, evict_gu, jouter, q0_wts=q0_wts)

            stage = {}

            def evict_dn(f, t, ps):
                if f not in stage:
                    stage[f] = outp.tile([P, T], BF, tag="o", name="stage")
                if grow is None:
                    nc.scalar.copy(
                        out=stage[f][:, t * TT:(t + 1) * TT], in_=ps[:])
                else:
                    nc.vector.tensor_mul(
                        out=stage[f][:, t * TT:(t + 1) * TT],
                        in0=ps[:],
                        in1=grow[:, t * TT:(t + 1) * TT],
                    )
                nc.sync.dma_start(
                    out=out_dram[f * P:(f + 1) * P, t * TT:(t + 1) * TT],
                    in_=stage[f][:, t * TT:(t + 1) * TT],
                )

            layer2(w2_dram, h, evict_dn)

        # ---- PE warm-up: throwaway matmuls keep the PE busy (and open the
        # HAM clock-gate, 1.2 -> 2.4 GHz) while the first chunks land.
        wu_a = tmpp.tile([P, P], BF, tag="wu_a", name="wu_a")
        wu_b = tmpp.tile([P, TT], BF, tag="wu_b", name="wu_b")
        nc.vector.memset(wu_a[:], 0.0)
        nc.vector.memset(wu_b[:], 0.0)
        wu_ps = psum.tile([P, TT], F32, tag="ps", name="wu_ps")
        for _ in range(WARMUP):
            nc.tensor.matmul(wu_ps[:], lhsT=wu_a[:], rhs=wu_b[:],
                             start=True, stop=True)

        # ---- shared expert on this core's token slice ----
        # Hand-ordered preamble: x chunk 0 streams on the scalar ring in
        # parallel with the sync ring, whose issue order matches the q0
        # k-outer consumption order exactly (w s0 halves, xs c1, w s1,
        # xs c2, xs c3), so the PE never waits once the first 1MB lands.
        xs = [xpool.tile([P, 4, T], BF, tag="x", name="xt")
              for _ in range(NXC)]
        wq0 = [w1pool.tile([P, 8, FQ], BF, tag="w1", name="w1t")
               for _ in range(2)]
        nc.scalar.dma_start(out=xs[0][:, 0:1, :], in_=xs_p[0][:, 0:T])
        nc.scalar.dma_start(out=xs[0][:, 1:2, :], in_=xs_p[0][:, T:2 * T])
        nc.scalar.dma_start(out=xs[0][:, 2:4, :], in_=xs_p[0][:, 2 * T:])
        nc.sync.dma_start(out=wq0[0][:, 0:2, :], in_=w1s_p[0][0][:, 0:2 * FQ])
        nc.sync.dma_start(out=wq0[0][:, 2:4, :], in_=w1s_p[0][0][:, 2 * FQ:4 * FQ])
        nc.sync.dma_start(out=wq0[0][:, 4:8, :], in_=w1s_p[0][0][:, 4 * FQ:])
        nc.sync.dma_start(out=xs[1][:], in_=xs_p[1])
        nc.sync.dma_start(out=wq0[1][:], in_=w1s_p[0][1])
        nc.sync.dma_start(out=xs[2][:], in_=xs_p[2])
        nc.sync.dma_start(out=xs[3][:], in_=xs_p[3])
        unit(xs, w1s_p, w2s_p, ys_p, silu_first=False, jouter=True,
             q0_wts=wq0)

        # ---- routed experts (2 per core) ----
        for e in range(EPC):
            xe = load_x(xr_p[e], nc.sync)
            grow = gpool.tile([P, T], F32, tag="g", name="grow")
            nc.scalar.dma_start(out=grow[:], in_=gat[e])
            unit(xe, w1e_p[e], w2e_p[e], yr_p[e], silu_first=True, grow=grow)

    return nc


# --------------------------------------------------------------------------
# Device execution wrappers
# --------------------------------------------------------------------------
def _make_in_maps(x_flat, gti, gating, gate_up_proj, down_proj,
                  shared_in_w, shared_out_w):
    import ml_dtypes

    bf16 = ml_dtypes.bfloat16

    w1s = _pack_w1(shared_in_w).astype(bf16)
    w2s = _pack_w2(shared_out_w).astype(bf16)
    w1e_all = [_pack_w1(gate_up_proj[e]).astype(bf16) for e in range(E)]
    w2e_all = [_pack_w2(down_proj[e]).astype(bf16) for e in range(E)]

    in_maps = []
    for c in range(NCORES):
        e0 = c * EPC
        xr = np.stack([
            _pack_x(np.ascontiguousarray(x_flat[gti[e].reshape(-1)].T))
            for e in range(e0, e0 + EPC)
        ])  # (EPC, NXC, P, 4*T)
        xs = _pack_x(np.ascontiguousarray(x_flat[c * TSH:(c + 1) * TSH].T))
        in_maps.append({
            "xs_p": xs.astype(bf16),
            "xr_p": xr.astype(bf16),
            "w1s_p": w1s,
            "w2s_p": w2s,
            "w1e_p": np.stack(w1e_all[e0:e0 + EPC]),
            "w2e_p": np.stack(w2e_all[e0:e0 + EPC]),
            "gat": np.ascontiguousarray(np.broadcast_to(
                gating[e0:e0 + EPC].reshape(EPC, 1, TE),
                (EPC, P, TE))).astype(np.float32),
        })
    return in_maps


def _run_numpy(in_maps):
    """Emulates the device math (bf16 inputs, fp32 accumulation)."""
    import ml_dtypes
    results = []
    for m in in_maps:
        def mlp(xp, w1p, w2p, silu_first):
            xT = _unpack_x(np.asarray(xp, np.float32))          # (DIM, T)
            wi = _unpack_w1(np.asarray(w1p, np.float32))        # (DIM, I2)
            wo = _unpack_w2(np.asarray(w2p, np.float32))        # (INNER, DIM)
            gu = wi.T @ xT                                       # (I2, T)
            a, b = gu[:INNER], gu[INNER:]
            silu = lambda v: v / (1.0 + np.exp(-v))
            h = (silu(a) * b) if silu_first else (a * silu(b))
            h = h.astype(ml_dtypes.bfloat16).astype(np.float32)
            return wo.T @ h                                      # (DIM, T)

        ys = mlp(m["xs_p"], m["w1s_p"], m["w2s_p"], False)
        yr = np.stack([
            mlp(m["xr_p"][e], m["w1e_p"][e], m["w2e_p"][e], True)
            * m["gat"][e][:1, :]
            for e in range(EPC)
        ])
        results.append({
            "ys_p": ys.astype(ml_dtypes.bfloat16),
            "yr_p": yr.astype(ml_dtypes.bfloat16),
        })
    return results, None


_NC_CACHE = {}


def _install_ntff_hook():
    """Provide antenv.axon_hooks (missing in this image) so
    run_bass_kernel_spmd(trace=True) can NTFF-profile via the axon .so."""
    import contextlib
    import ctypes
    import types

    name = "antenv.axon_hooks"
    if name in sys.modules:
        return
    try:
        import antenv.axon_hooks  # noqa: F401
        return
    except ImportError:
        pass
    so_path = "/opt/axon/libaxon_pjrt.so"
    if not os.path.exists(so_path):
        return
    lib = ctypes.CDLL(so_path)
    if not hasattr(lib, "axon_start_nrt_profile"):
        return
    lib.axon_start_nrt_profile.argtypes = [
        ctypes.POINTER(ctypes.c_int64), ctypes.c_size_t]
    lib.axon_start_nrt_profile.restype = ctypes.c_int64
    lib.axon_stop_nrt_profile.argtypes = [ctypes.c_char_p]
    lib.axon_stop_nrt_profile.restype = ctypes.c_int64

    @contextlib.contextmanager
    def _hook(output_dir, device_ids):
        import jax
        jax.devices()
        if device_ids:
            ids = (ctypes.c_int64 * len(device_ids))(*device_ids)
            rc = lib.axon_start_nrt_profile(ids, len(device_ids))
        else:
            rc = lib.axon_start_nrt_profile(None, 0)
        if rc != 0:
            raise RuntimeError(f"axon_start_nrt_profile rc={rc}")
        try:
            yield
        finally:
            n = lib.axon_stop_nrt_profile(str(output_dir).encode())
            print(f"profile: {n} file(s) written to {output_dir}",
                  file=sys.stderr)

    mod = types.ModuleType(name)
    mod._hook = _hook
    mod.set_axon_ntff_profile_hook = lambda h: setattr(mod, "_hook", h)
    mod.get_axon_ntff_profile_hook = lambda: mod._hook
    sys.modules[name] = mod


def _run_bass(in_maps):
    from concourse.bass_utils import run_bass_kernel_spmd

    if "nc" not in _NC_CACHE:
        _NC_CACHE["nc"] = _split_multi_waits(_strip_entry_barrier(_build_nc()))
    nc = _NC_CACHE["nc"]
    trace = os.environ.get("KERNEL_TRACE", "0") == "1"
    if trace:
        _install_ntff_hook()
    out = run_bass_kernel_spmd(nc, in_maps, list(range(NCORES)), trace=trace)
    if out.exec_time_ns is not None:
        print(f"HW exec time: {out.exec_time_ns} ns", flush=True)
        if out.mean_exec_time_ns is not None:
            print(f"HW mean exec time: {out.mean_exec_time_ns:.0f} ns", flush=True)
    return out.results, out.exec_time_ns


# --------------------------------------------------------------------------
# Public entry point
# --------------------------------------------------------------------------
def kernel(hidden_states, hidden_states_unmodulated, timestep, gate_w,
           gate_up_proj, down_proj, shared_in_w, shared_out_w):
    hidden_states = np.asarray(hidden_states, dtype=np.float32)
    x_flat = hidden_states.reshape(NTOK, DIM)

    gti, gating = _route(np.asarray(hidden_states_unmodulated),
                         np.asarray(timestep), np.asarray(gate_w))

    in_maps = _make_in_maps(
        x_flat, gti, gating,
        np.asarray(gate_up_proj, dtype=np.float32),
        np.asarray(down_proj, dtype=np.float32),
        np.asarray(shared_in_w, dtype=np.float32),
        np.asarray(shared_out_w, dtype=np.float32),
    )

    if _BACKEND == "numpy":
        results, _ = _run_numpy(in_maps)
    else:
        results, _ = _run_bass(in_maps)

    # ---- combine on host ----
    out_flat = np.empty((NTOK, DIM), np.float32)
    for c in range(NCORES):
        out_flat[c * TSH:(c + 1) * TSH] = np.asarray(
            results[c]["ys_p"], np.float32).T
    for c in range(NCORES):
        yr = np.asarray(results[c]["yr_p"], np.float32)  # (EPC, DIM, TE)
        for ei in range(EPC):
            e = c * EPC + ei
            rows = yr[ei].T  # (TE, DIM) in (b, slot) order
            for b in range(BS):
                idx = gti[e, b]
                out_flat[idx] += rows[b * CAP:(b + 1) * CAP]
    return out_flat.reshape(BS, SLEN, DIM)
